# revision 9
# baseline (speedup 1.0000x reference)
"""Trainium2 Bass kernel for nn_BaseEmbedder (retrieval_knn).

For each of 4096 query embeddings: find the 5 nearest of 65536 db embeddings
(Euclidean) and produce the inverse-distance-weighted sum of their auxiliary
features.  SPMD on 8 NeuronCores: queries sharded 512/core, db+aux replicated.

v2 design (per core, 512 queries = 4 q-tiles of 128 partitions):
  - db resident: all 65536 db columns live in SBUF as 32 bf16 tiles
    [128, 1024] (super 2i on partitions 0:34, super 2i+1 on 64:98), loaded
    once and reused by all 4 q-tiles.
  - Scan (bf16): negS[q,j] = q.x_j - 0.5|x_j|^2 via K=34 augmented bf16
    matmuls (rows 32/33 = bias hi/lo).  Per round i (32 rounds/q-tile) four
    512-col matmuls fill one PSUM tile [128, 2048]: cols 0:1024 = super 2i
    (PE rows 0:34), cols 1024:2048 = super 2i+1 (PE rows 64:98).
  - 8:1 max-fold to zfold[8192] bf16, split across engines per round:
      P1 rounds: Act evacuates psum[:,1024:2048]->bf16, DVE
                 tensor_tensor(max)(psum f32, evac bf16) -> L1.
      P2 rounds: Act evacuates all 2048 -> bf16, DVE folds bf16 (2x mode).
      P3 rounds: Act evacuates all 2048 -> bf16, GpSimd folds bf16.
    L2a/L2b on DVE (bf16 2x), L3 on GpSimd; zfold overlays L1buf[:, :8192].
  - top-8 fold slots: max8 + find_index8 over the 8192-wide bf16 window.
    Slot s (i0=s//1024, u=s%1024) covers db rows 2048*(i0+8*m3)+1024*h+u.
  - Exact f32 refinement: pair_table[s] = 8x[x(32) aux(32)] + 8x|x|^2
    (f32, host-prepared) gathered per winning slot via indirect DMA; exact
    top-5 by threshold; weights 1/(d+eps); weighted aux sum.

The bf16 scan only nominates candidate slots; selection + weights are exact
f32, so the result matches the f32 reference to ~1e-6.
"""

import numpy as np
import ml_dtypes

from concourse import bass, mybir
from concourse.tile import TileContext
from concourse.bass_utils import run_bass_kernel_spmd

F32 = mybir.dt.float32
BF16 = mybir.dt.bfloat16
U32 = mybir.dt.uint32
I32 = mybir.dt.int32

N_CORES = 8
NQ = 4096
NDB = 65536
D = 32
DAUG = 34   # 32 dims + bias row + bias-residual row (bf16 split)
K = 5
EPS = 1e-6

NQ_CORE = NQ // N_CORES          # 512
RG_B = 64                        # partition base of the second PE row-group
N_ROUND = 32                     # rounds per q-tile, 2048 db cols each
FOLD_W = 8192                    # zfold width (65536 / 8)
NSLOT = 8                        # gathered slots per query
PV = 8 * (2 * D) + 8             # pair row: 8x[x aux] + 8x |x|^2 = 520 f32

# round classes: how each round's 1024 L1 outputs are produced
#   1 = Act evac B-half,  DVE TT(psumA f32, evac bf16)
#   2 = Act evac both,    DVE TT(bf16, bf16)   (2x mode)
# (GPSIMD cannot run TensorTensor or InstPool, and cannot touch PSUM, so the
#  fold is strictly DVE+Act; the split below balances the two engines.)
ROUND_CLS = ([1] * 2 + [2] * 30)

# process rounds 8..31 first so the next q-tile's early L1 writes (which
# overlay the previous q-tile's zfold window l1[:, 0:8192]) land after that
# window has been consumed by max8/find_index8.
ROUND_ORDER = list(range(8, N_ROUND)) + list(range(8))


def build_nc(nq_core=NQ_CORE, ndb=NDB):
    n_qt = nq_core // 128
    assert ndb == N_ROUND * 2048

    nc = bass.Bass()
    qT = nc.declare_dram_parameter("qT_aug", [DAUG, nq_core], BF16, isOutput=False)
    qf = nc.declare_dram_parameter("qf", [nq_core, D], F32, isOutput=False)
    qsq = nc.declare_dram_parameter("qsq", [nq_core, 1], F32, isOutput=False)
    dbT = nc.declare_dram_parameter("dbT_aug", [DAUG, ndb], BF16, isOutput=False)
    pairt = nc.declare_dram_parameter("pair_table", [FOLD_W, PV], F32,
                                      isOutput=False)
    out = nc.declare_dram_parameter("out", [nq_core, D], F32, isOutput=True)

    with TileContext(nc) as tc:
        with (
            tc.tile_pool(name="db", bufs=1) as dbp,
            tc.tile_pool(name="l1", bufs=1) as l1p,
            tc.tile_pool(name="l2", bufs=1) as l2p,
            tc.tile_pool(name="ev", bufs=2) as evp,
            tc.tile_pool(name="ps", bufs=2, space="PSUM") as psp,
            tc.tile_pool(name="sm", bufs=2) as sp,
            tc.tile_pool(name="g", bufs=1) as gp,
            tc.tile_pool(name="pr", bufs=1) as prp,
        ):
            # ---- load db into SBUF once (32 tiles, 2 supers each) ----
            dtiles = []
            for i in range(N_ROUND):
                dt = dbp.tile([128, 1024], BF16, tag=f"db{i}")
                nc.sync.dma_start(out=dt[0:DAUG, :],
                                  in_=dbT[:, 2048 * i:2048 * i + 1024])
                nc.sync.dma_start(out=dt[RG_B:RG_B + DAUG, :],
                                  in_=dbT[:, 2048 * i + 1024:2048 * i + 2048])
                dtiles.append(dt)

            for t in range(n_qt):
                qt = sp.tile([128, 128], BF16, tag="qt")
                nc.sync.dma_start(out=qt[0:DAUG, :],
                                  in_=qT[:, t * 128:(t + 1) * 128])
                nc.sync.dma_start(out=qt[RG_B:RG_B + DAUG, :],
                                  in_=qT[:, t * 128:(t + 1) * 128])
                qs = sp.tile([128, 1], F32, tag="qs")
                nc.sync.dma_start(out=qs[:], in_=qsq[t * 128:(t + 1) * 128, :])
                qft = sp.tile([128, D], F32, tag="qft")
                nc.sync.dma_start(out=qft[:], in_=qf[t * 128:(t + 1) * 128, :])

                # L1 fold buffer; zfold overlays its first 8192 columns after
                # L2a/L2b have consumed all of L1.
                l1 = l1p.tile([128, 32768], BF16, tag="l1")
                l2 = l2p.tile([128, 16384], BF16, tag="l2")

                for ri, i in enumerate(ROUND_ORDER):
                    ps = psp.tile([128, 2048], F32, tag="ps")
                    for m in range(2):
                        sl = slice(m * 512, (m + 1) * 512)
                        nc.tensor.matmul(out=ps[:, sl],
                                         lhsT=qt[0:DAUG, :],
                                         rhs=dtiles[i][0:DAUG, sl],
                                         start=True, stop=True,
                                         tile_position=(0, 0))
                    for m in range(2):
                        sl = slice(m * 512, (m + 1) * 512)
                        nc.tensor.matmul(out=ps[:, 1024 + m * 512:1024 + (m + 1) * 512],
                                         lhsT=qt[RG_B:RG_B + DAUG, :],
                                         rhs=dtiles[i][RG_B:RG_B + DAUG, sl],
                                         start=True, stop=True,
                                         tile_position=(RG_B, 0))
                    cls = ROUND_CLS[ri]
                    ldst = l1[:, 1024 * i:1024 * (i + 1)]
                    if cls == 1:
                        ev = evp.tile([128, 1024], BF16, tag="ev")
                        nc.scalar.copy(out=ev[:], in_=ps[:, 1024:2048])
                        nc.vector.tensor_tensor(out=ldst, in0=ps[:, 0:1024],
                                                in1=ev[:],
                                                op=mybir.AluOpType.max)
                    else:
                        ev = evp.tile([128, 2048], BF16, tag="ev2")
                        nc.scalar.copy(out=ev[:], in_=ps[:])
                        nc.vector.tensor_tensor(out=ldst, in0=ev[:, 0:1024],
                                                in1=ev[:, 1024:2048],
                                                op=mybir.AluOpType.max)

                # L2a/L2b (DVE bf16 2x), L3 (GpSimd), zfold = l1[:, 0:8192]
                nc.vector.tensor_tensor(out=l2[:, 0:8192],
                                        in0=l1[:, 0:8192],
                                        in1=l1[:, 8192:16384],
                                        op=mybir.AluOpType.max)
                nc.vector.tensor_tensor(out=l2[:, 8192:16384],
                                        in0=l1[:, 16384:24576],
                                        in1=l1[:, 24576:32768],
                                        op=mybir.AluOpType.max)
                zfold = l1[:, 0:8192]
                nc.vector.tensor_tensor(out=zfold, in0=l2[:, 0:8192],
                                        in1=l2[:, 8192:16384],
                                        op=mybir.AluOpType.max)

                # top-8 fold slots
                w8 = sp.tile([128, 8], BF16, tag="w8")
                nc.vector.max(out=w8[:], in_=zfold)
                pos = sp.tile([128, 8], U32, tag="pos")
                nc.vector.max_index(out=pos[:], in_max=w8[:], in_values=zfold)
                ji = sp.tile([128, 8], I32, tag="ji")
                nc.vector.tensor_copy(ji[:], pos[:])

                gxa = gp.tile([128, NSLOT, PV], F32, tag="gxa")
                for i in range(NSLOT):
                    nc.gpsimd.indirect_dma_start(
                        out=gxa[:, i, :], out_offset=None, in_=pairt[:],
                        in_offset=bass.IndirectOffsetOnAxis(
                            ap=ji[:, i:i + 1], axis=0))

                # ---- exact f32 refinement over the 64 candidates ----
                base = gxa[:, :, 0:16 * D].rearrange("p c (h v) -> p c h v", h=8)
                gx = base[:, :, :, 0:D]
                ga = base[:, :, :, D:2 * D]
                xsq = gxa[:, :, 16 * D:16 * D + 8]          # [128, 8, 8]
                pr = prp.tile([128, NSLOT, 8, D], F32, tag="pr")
                nc.vector.tensor_tensor(
                    out=pr[:], in0=gx,
                    in1=qft[:].unsqueeze(1).unsqueeze(1)
                              .to_broadcast([128, NSLOT, 8, D]),
                    op=mybir.AluOpType.mult)
                dots = sp.tile([128, NSLOT, 8], F32, tag="dots")
                nc.vector.tensor_reduce(out=dots[:], in_=pr[:],
                                        axis=mybir.AxisListType.X,
                                        op=mybir.AluOpType.add)
                ncand = NSLOT * 8
                # neg2 = 2*dots - xsq  (dsq = qsq - neg2)
                neg2 = sp.tile([128, ncand], F32, tag="neg2")
                nc.vector.scalar_tensor_tensor(
                    out=neg2[:].rearrange("p (c h) -> p c h", h=8),
                    in0=dots[:], scalar=2.0, in1=xsq,
                    op0=mybir.AluOpType.mult, op1=mybir.AluOpType.subtract)
                t8 = sp.tile([128, 8], F32, tag="t8")
                nc.vector.max(out=t8[:], in_=neg2[:])
                mask = sp.tile([128, ncand], F32, tag="mask")
                nc.vector.tensor_scalar(mask[:], neg2[:], t8[:, 4:5], None,
                                        op0=mybir.AluOpType.is_ge)
                dsq = sp.tile([128, ncand], F32, tag="dsq")
                nc.vector.tensor_scalar(dsq[:], neg2[:], -1.0, qs[:, 0:1],
                                        op0=mybir.AluOpType.mult,
                                        op1=mybir.AluOpType.add)
                nc.vector.tensor_scalar_max(dsq[:], dsq[:], 0.0)
                dist = sp.tile([128, ncand], F32, tag="dist")
                nc.scalar.sqrt(out=dist[:], in_=dsq[:])
                nc.vector.tensor_scalar_add(dist[:], dist[:], EPS)
                rec = sp.tile([128, ncand], F32, tag="rec")
                nc.vector.reciprocal(out=rec[:], in_=dist[:])
                wgt = sp.tile([128, ncand], F32, tag="wgt")
                nc.vector.tensor_tensor(out=wgt[:], in0=rec[:], in1=mask[:],
                                        op=mybir.AluOpType.mult)
                wsum = sp.tile([128, 1], F32, tag="wsum")
                nc.vector.tensor_reduce(out=wsum[:], in_=wgt[:],
                                        axis=mybir.AxisListType.X,
                                        op=mybir.AluOpType.add)
                winv = sp.tile([128, 1], F32, tag="winv")
                nc.vector.reciprocal(out=winv[:], in_=wsum[:])

                prod = prp.tile([128, NSLOT, 8, D], F32, tag="pr")
                nc.vector.tensor_tensor(
                    out=prod[:], in0=ga,
                    in1=wgt[:].rearrange("p (c h) -> p c h", h=8).unsqueeze(-1)
                              .to_broadcast([128, NSLOT, 8, D]),
                    op=mybir.AluOpType.mult)
                acc = sp.tile([128, D], F32, tag="accr")
                nc.vector.tensor_reduce(
                    out=acc[:],
                    in_=prod[:].rearrange("p i h a -> p a (i h)"),
                    axis=mybir.AxisListType.X, op=mybir.AluOpType.add)
                outt = sp.tile([128, D], F32, tag="outt")
                nc.vector.tensor_scalar(outt[:], acc[:], winv[:, 0:1], None,
                                        op0=mybir.AluOpType.mult)
                nc.sync.dma_start(out=out[t * 128:(t + 1) * 128, :], in_=outt[:])

    split_multi_waits(nc)
    return nc


def split_multi_waits(nc):
    """The walrus build in this container supports a single sync-wait per
    instruction; Tile's tail drain carries one wait per live proc.  Split
    any multi-wait instruction into single-wait NoOps ahead of it."""
    for f in nc.m.functions:
        for blk in f.blocks:
            newinsts = []
            for ins in blk.instructions:
                si = ins.sync_info
                if si is not None and si.on_wait and len(si.on_wait) > 1:
                    waits = list(si.on_wait)
                    for k, w in enumerate(waits[:-1]):
                        nop = mybir.InstNoOp(name=f"{ins.name}-ws{k}", ins=[],
                                             outs=[])
                        nop.engine = ins.engine
                        nop.sync_info = mybir.SyncInfo(on_wait=[w], on_update=[])
                        newinsts.append(nop)
                    ins.sync_info = mybir.SyncInfo(on_wait=[waits[-1]],
                                                   on_update=list(si.on_update))
                newinsts.append(ins)
            blk.instructions = newinsts


def make_in_maps(embedding_features, db_embedding, auxiliary_features):
    q = np.ascontiguousarray(np.asarray(embedding_features, dtype=np.float32))
    db = np.ascontiguousarray(np.asarray(db_embedding, dtype=np.float32))
    aux = np.ascontiguousarray(np.asarray(auxiliary_features, dtype=np.float32))
    ndb = db.shape[0]
    nq_core = q.shape[0] // N_CORES
    bf = ml_dtypes.bfloat16
    bias = -0.5 * (db * db).sum(1)                      # exact f32
    b_hi = bias.astype(bf).astype(np.float32)
    b_lo = (bias - b_hi).astype(bf)
    dbT_aug = np.ascontiguousarray(np.concatenate(
        [db.T.astype(bf), b_hi.astype(bf)[None, :], b_lo[None, :]], axis=0,
        dtype=bf))
    # slot s = 1024*i0 + u (i0 = 0..7) covers db rows
    #   2048*(i0 + 8*m3) + 1024*h + u   for m3 = 0..3, h = 0..1
    s = np.arange(FOLD_W)
    i0 = s // 1024
    u = s % 1024
    dbsq = (db * db).sum(1)
    pair_table = np.zeros((FOLD_W, PV), np.float32)
    for m3 in range(4):
        for h in range(2):
            m = 2 * m3 + h
            jm = 2048 * (i0 + 8 * m3) + 1024 * h + u
            pair_table[:, 2 * m * D:(2 * m + 1) * D] = db[jm]
            pair_table[:, (2 * m + 1) * D:(2 * m + 2) * D] = aux[jm]
            pair_table[:, 16 * D + m] = dbsq[jm]
    pair_table = np.ascontiguousarray(pair_table)
    in_maps = []
    for c in range(N_CORES):
        qsl = q[c * nq_core:(c + 1) * nq_core]
        qT_aug = np.ascontiguousarray(np.concatenate(
            [qsl.T.astype(bf), np.ones((2, nq_core), bf)], axis=0, dtype=bf))
        qsq = np.ascontiguousarray((qsl * qsl).sum(1).reshape(nq_core, 1)
                                   ).astype(np.float32)
        in_maps.append({"qT_aug": qT_aug, "qf": qsl, "qsq": qsq,
                        "dbT_aug": dbT_aug, "pair_table": pair_table})
    return in_maps


_NC_CACHE = {}


def get_nc(nq_core=NQ_CORE, ndb=NDB):
    key = (nq_core, ndb)
    if key not in _NC_CACHE:
        _NC_CACHE[key] = build_nc(nq_core, ndb)
    return _NC_CACHE[key]


def kernel(embedding_features, db_embedding, auxiliary_features):
    in_maps = make_in_maps(embedding_features, db_embedding, auxiliary_features)
    nc = get_nc()
    res = run_bass_kernel_spmd(nc, in_maps, list(range(N_CORES)))
    return np.concatenate([res.results[c]["out"] for c in range(N_CORES)],
                          axis=0).astype(np.float32)
